# revision 8
# baseline (speedup 1.0000x reference)
"""2-layer GAT on 8 trn2 cores — stream-based (no on-device gathers).

Strategy:
  - Host bin-packs the 50000 nodes into 8 cores x 392 groups of 16 slots,
    balancing in-degree so every group has <= 256 in-edges (CPG=2 chunks of
    128 edge-lanes per group). The structure is identical across cores and
    groups -> one fully static SPMD program per launch.
  - K1 (per core): h1cat = x_shard @ [W1 c-major | W1@As | W1@Ad]  (bf16).
  - host: assembles per-edge LINEAR streams in chunk-lane order
    (dst-sharded): hs[e] = h1[src] (256 bf16, c-major),
    es[e] = [a_src | a_dst | dl | 0] (dl = in-group dst index, -1 pads).
  - K2: ee = exp(lrelu(asrc+adst)); 16-wide one-hot from dl via iota
    is_equal; mt = h1 * ee (DVE/Pool split); 16-row matmuls accumulate
    [msg | ee] into 8 psum regions per [128,264] tile; divide by denom,
    +bias1, ELU, transpose, @W2cat -> h2cat = [h2 | a2src | a2dst].
  - host: hs2[e] = [h2[src] | 1 | 0], es2[e] = [a2src | a2dst | dl | 0].
  - K3: ee2 folded into the one-hot; matmul rhs [h2|1]; divide, +bias2.
Softmax max-subtraction is skipped: alpha = exp(e)/sum(exp(e)) is exact and
e is O(5), safe in f32/bf16.
"""

import heapq
import sys

sys.path.insert(0, "/opt/trn_rl_repo")

from contextlib import ExitStack

import ml_dtypes
import numpy as np

from concourse import bacc, bass, mybir
from concourse.bass_utils import run_bass_kernel_spmd
from concourse.masks import make_identity
from concourse.tile import TileContext

P = 128
N = 50000
NCORE = 8
GS = 32  # nodes per group (one-hot width / psum region rows; PE quadrant=32)
GPC = 196  # groups per core
RPT = 4  # psum regions (groups) per [128, *] psum tile
SLOTS = GPC * GS  # 6272 node slots per core
NT = GPC // RPT  # 49 psum tiles
IN_C = 128
HEADS = 8
HID = 32
C1 = HEADS * HID  # 256
OUT_C = 64
CAT1 = C1 + 2 * HEADS  # 272
CAT2 = OUT_C + 2  # 66
MW = C1 + HEADS  # 264  (msg | ee)
ES1 = 18  # estream1 cols: asrc(8) adst(8) dl pad
HS2 = 66  # hstream2 cols: h2(64) one pad
ES2 = 4  # estream2 cols: a2src a2dst dl pad
NEG_SLOPE = 0.2
EPS = 1e-16
C_DVE = 26  # c-columns (of 32) of the mt multiply on DVE
C_POOL = HID - C_DVE  # rest on Pool

f32 = mybir.dt.float32
bf16 = mybir.dt.bfloat16
i32 = mybir.dt.int32
AF = mybir.ActivationFunctionType
OP = mybir.AluOpType
BF = ml_dtypes.bfloat16

CORE_IDS = list(range(NCORE))

# c-major col j = c*HEADS + h  <-  h-major col h*HID + c
PERM_CM = (np.arange(C1) % HEADS) * HID + (np.arange(C1) // HEADS)


def _pack_nodes(deg):
    """LPT bin-packing of nodes into NCORE*GPC groups of <=GS nodes each,
    then swap-repair so every group in-degree sum fits CPG=4 chunks."""
    NG = NCORE * GPC
    order = np.argsort(-deg, kind="stable")
    heap = [(0, 0, g) for g in range(NG)]
    heapq.heapify(heap)
    bins = [[] for _ in range(NG)]
    sums = np.zeros(NG, np.int64)
    for n in order:
        s, cnt, g = heapq.heappop(heap)
        bins[g].append(int(n))
        s += int(deg[n])
        sums[g] = s
        if cnt + 1 < GS:
            heapq.heappush(heap, (s, cnt + 1, g))
    target = 4 * P
    for _ in range(50000):
        gmax = int(np.argmax(sums))
        over = int(sums[gmax]) - target
        if over <= 0:
            break
        gmin = int(np.argmin(sums))
        best = None
        for a in bins[gmax]:
            for b in bins[gmin]:
                d = int(deg[a]) - int(deg[b])
                if d <= 0 or int(sums[gmin]) + d > target:
                    continue
                key = (0, d) if d >= over else (1, -d)
                if best is None or key < best[0]:
                    best = (key, a, b)
        if best is None:
            break
        _, a, b = best
        bins[gmax].remove(a)
        bins[gmin].remove(b)
        bins[gmax].append(b)
        bins[gmin].append(a)
        sums[gmax] += deg[b] - deg[a]
        sums[gmin] += deg[a] - deg[b]
    cpg = max(4, int(-(-int(sums.max()) // P)))
    return bins, cpg


def _prep_graph(edge_index):
    src = np.asarray(edge_index[0], np.int64)
    dst = np.asarray(edge_index[1], np.int64)
    E = src.shape[0]
    deg = np.bincount(dst, minlength=N)
    bins, cpg = _pack_nodes(deg)
    EPG = cpg * P
    ESLOT = GPC * EPG

    slot_node = np.full((NCORE, SLOTS), -1, np.int64)
    node_core = np.zeros(N, np.int64)
    node_slot = np.zeros(N, np.int64)
    for g, nodes in enumerate(bins):
        c, gl = g // GPC, g % GPC
        for i, n in enumerate(nodes):
            slot_node[c, gl * GS + i] = n
            node_core[n] = c
            node_slot[n] = gl * GS + i
    node_flat = node_core * SLOTS + node_slot  # row in stacked per-core output

    gglob = node_core[dst] * GPC + node_slot[dst] // GS
    order_e = np.argsort(gglob, kind="stable")
    cnt = np.bincount(gglob, minlength=NCORE * GPC)
    off = np.concatenate([[0], np.cumsum(cnt)])
    pos = np.arange(E) - off[gglob[order_e]]
    assert pos.max() < EPG, "group over capacity after packing"
    se, de = src[order_e], dst[order_e]
    cc = node_core[de]
    eslot = (node_slot[de] // GS) * EPG + pos

    esrc = np.full((NCORE, ESLOT), -1, np.int64)
    ednode = np.full((NCORE, ESLOT), -1, np.int64)
    edl = np.full((NCORE, ESLOT), -1, np.int64)
    esrc[cc, eslot] = se
    ednode[cc, eslot] = de
    edl[cc, eslot] = node_slot[de] % GS
    return {
        "cpg": cpg,
        "slot_node": slot_node,
        "node_flat": node_flat,
        "esrc": esrc,
        "ednode": ednode,
        "edl": edl,
    }


def _build_k1():
    nc = bacc.Bacc("TRN2", target_bir_lowering=False)
    XT = nc.dram_tensor("xT", [P, SLOTS], bf16, kind="ExternalInput")
    W = nc.dram_tensor("w1cat", [P, CAT1], bf16, kind="ExternalInput")
    OUT = nc.dram_tensor("h1cat", [P, NT, CAT1], bf16, kind="ExternalOutput")
    with TileContext(nc) as tc, ExitStack() as ctx:
        sb = ctx.enter_context(tc.tile_pool(name="sb", bufs=2))
        con = ctx.enter_context(tc.tile_pool(name="con", bufs=1))
        ps = ctx.enter_context(tc.tile_pool(name="ps", bufs=4, space="PSUM"))
        w = con.tile([P, CAT1], bf16)
        nc.sync.dma_start(out=w[:], in_=W[:])
        xt = con.tile([P, SLOTS], bf16)
        for q in range(4):
            w4 = SLOTS // 4
            nc.sync.dma_start(
                out=xt[:, q * w4 : (q + 1) * w4], in_=XT[:, q * w4 : (q + 1) * w4]
            )
        for g in range(7):
            ot = sb.tile([P, 7, CAT1], bf16)
            for u in range(7):
                t = g * 7 + u
                pt = ps.tile([P, 512], f32)
                nc.tensor.matmul(
                    pt[:, 0:CAT1], lhsT=xt[:, t * P : (t + 1) * P], rhs=w[:],
                    start=True, stop=True,
                )
                if u % 2 == 0:
                    nc.vector.tensor_copy(out=ot[:, u, :], in_=pt[:, 0:CAT1])
                else:
                    nc.scalar.activation(out=ot[:, u, :], in_=pt[:, 0:CAT1], func=AF.Copy)
            nc.sync.dma_start(out=OUT[:, g * 7 : (g + 1) * 7, :], in_=ot[:])
    nc.compile()
    return nc


def _build_k2(cpg):
    TPC = RPT * cpg  # chunks per psum tile
    NCH = GPC * cpg
    CD = C_DVE * HEADS
    nc = bacc.Bacc("TRN2", target_bir_lowering=False)
    HS = nc.dram_tensor("hs", [P, NCH, C1], bf16, kind="ExternalInput")
    ES = nc.dram_tensor("es", [P, NCH, ES1], bf16, kind="ExternalInput")
    W2c = nc.dram_tensor("w2cat", [P, 2 * CAT2], bf16, kind="ExternalInput")
    B1 = nc.dram_tensor("bias1", [P, C1], bf16, kind="ExternalInput")
    OUT = nc.dram_tensor("h2cat", [P, NT, CAT2], bf16, kind="ExternalOutput")
    with TileContext(nc) as tc, ExitStack() as ctx:
        con = ctx.enter_context(tc.tile_pool(name="con", bufs=1))
        hpool = ctx.enter_context(tc.tile_pool(name="h", bufs=3))
        epool = ctx.enter_context(tc.tile_pool(name="e", bufs=3))
        spool = ctx.enter_context(tc.tile_pool(name="s", bufs=3))
        mpool = ctx.enter_context(tc.tile_pool(name="m", bufs=3))
        tpool = ctx.enter_context(tc.tile_pool(name="t", bufs=2))
        psagg = ctx.enter_context(tc.tile_pool(name="pa", bufs=4, space="PSUM"))
        pstp = ctx.enter_context(tc.tile_pool(name="pt", bufs=2, space="PSUM"))
        psmm = ctx.enter_context(tc.tile_pool(name="pm", bufs=2, space="PSUM"))

        w2t = con.tile([P, 2 * CAT2], bf16)
        nc.sync.dma_start(out=w2t[:], in_=W2c[:])
        b1t = con.tile([P, C1], bf16)
        nc.sync.dma_start(out=b1t[:], in_=B1[:])
        ident = con.tile([P, P], bf16)
        make_identity(nc, ident[:])
        iota_i = con.tile([P, GS], i32)
        nc.gpsimd.iota(iota_i[:], pattern=[[1, GS]], base=0, channel_multiplier=0)
        iota_b = con.tile([P, GS], bf16)
        nc.vector.tensor_copy(out=iota_b[:], in_=iota_i[:])
        iota_e = con.tile([P, GS, TPC], bf16)
        nc.vector.tensor_copy(
            out=iota_e[:], in_=iota_b[:][:, :, None].to_broadcast([P, GS, TPC])
        )

        stage = [None]
        for t in range(NT):
            c0 = t * TPC
            gt = hpool.tile([P, TPC, C1], bf16, tag="gt")
            nc.sync.dma_start(out=gt[:], in_=HS[:, c0 : c0 + TPC, :])
            es_t = epool.tile([P, TPC, ES1], bf16, tag="es")
            nc.sync.dma_start(out=es_t[:], in_=ES[:, c0 : c0 + TPC, :])
            dlt = epool.tile([P, TPC], bf16, tag="dl")
            nc.vector.tensor_copy(out=dlt[:], in_=es_t[:, :, 16])
            ea = epool.tile([P, TPC, HEADS], bf16, tag="ea")
            nc.vector.tensor_tensor(
                out=ea[:], in0=es_t[:, :, 0:8], in1=es_t[:, :, 8:16], op=OP.add
            )
            eb = epool.tile([P, TPC, HEADS], bf16, tag="eb")
            nc.vector.tensor_scalar(
                out=eb[:], in0=ea[:], scalar1=NEG_SLOPE, scalar2=None, op0=OP.mult
            )
            nc.vector.tensor_tensor(out=eb[:], in0=ea[:], in1=eb[:], op=OP.max)
            eet = epool.tile([P, TPC, HEADS], bf16, tag="eet")
            nc.scalar.activation(out=eet[:], in_=eb[:], func=AF.Exp)
            st2 = spool.tile([P, GS, TPC], bf16, tag="st2")
            nc.vector.tensor_tensor(
                out=st2[:],
                in0=dlt[:][:, None, :].to_broadcast([P, GS, TPC]),
                in1=iota_e[:],
                op=OP.is_equal,
            )
            mt = mpool.tile([P, TPC, MW], bf16, tag="mt")
            nc.vector.tensor_tensor(
                out=mt[:, :, 0:CD].rearrange("p a (c h) -> p a c h", h=HEADS),
                in0=gt[:, :, 0:CD].rearrange("p a (c h) -> p a c h", h=HEADS),
                in1=eet[:][:, :, None, :].to_broadcast([P, TPC, C_DVE, HEADS]),
                op=OP.mult,
            )
            if C_POOL:
                nc.gpsimd.tensor_tensor(
                    out=mt[:, :, CD:C1].rearrange("p a (c h) -> p a c h", h=HEADS),
                    in0=gt[:, :, CD:C1].rearrange("p a (c h) -> p a c h", h=HEADS),
                    in1=eet[:][:, :, None, :].to_broadcast([P, TPC, C_POOL, HEADS]),
                    op=OP.mult,
                )
            nc.vector.tensor_copy(out=mt[:, :, C1:MW], in_=eet[:])
            pagg = psagg.tile([P, 512], f32)
            for r in range(RPT):
                for j in range(cpg):
                    ch = r * cpg + j
                    nc.tensor.matmul(
                        pagg[r * GS : (r + 1) * GS, 0:MW],
                        lhsT=st2[:, :, ch],
                        rhs=mt[:, ch, :],
                        start=(j == 0),
                        stop=(j == cpg - 1),
                        tile_position=(0, r * GS),
                    )
            dent = tpool.tile([P, HEADS], f32, tag="dent")
            nc.vector.tensor_scalar(
                out=dent[:], in0=pagg[:, C1:MW], scalar1=EPS, scalar2=None, op0=OP.add
            )
            rec = tpool.tile([P, HEADS], f32, tag="rec")
            nc.vector.reciprocal(out=rec[:], in_=dent[:])
            o1 = tpool.tile([P, C1], bf16, tag="o1")
            nc.vector.tensor_tensor(
                out=o1[:].rearrange("p (c h) -> p c h", h=HEADS),
                in0=pagg[:, 0:C1].rearrange("p (c h) -> p c h", h=HEADS),
                in1=rec[:][:, None, :].to_broadcast([P, HID, HEADS]),
                op=OP.mult,
            )
            nc.gpsimd.tensor_tensor(out=o1[:], in0=o1[:], in1=b1t[:], op=OP.add)
            rneg = tpool.tile([P, C1], bf16, tag="rneg")
            nc.scalar.activation(out=rneg[:], in_=o1[:], func=AF.Relu, scale=-1.0)
            texp = tpool.tile([P, C1], bf16, tag="texp")
            nc.scalar.activation(out=texp[:], in_=rneg[:], func=AF.Exp, scale=-1.0)
            tpos = tpool.tile([P, C1], bf16, tag="tpos")
            nc.scalar.activation(out=tpos[:], in_=o1[:], func=AF.Relu)
            helu = tpool.tile([P, C1], bf16, tag="helu")
            nc.vector.scalar_tensor_tensor(
                out=helu[:], in0=texp[:], scalar=-1.0, in1=tpos[:],
                op0=OP.add, op1=OP.add,
            )
            p2 = psmm.tile([P, 512], f32)
            for k in range(2):
                tp = pstp.tile([P, 1024], bf16)
                nc.tensor.transpose(
                    out=tp[:, 0:P], in_=helu[:, k * P : (k + 1) * P], identity=ident[:]
                )
                hT = tpool.tile([P, P], bf16, tag="hT")
                nc.vector.tensor_copy(out=hT[:], in_=tp[:, 0:P])
                nc.tensor.matmul(
                    p2[:, 0:CAT2], lhsT=hT[:], rhs=w2t[:, k * CAT2 : (k + 1) * CAT2],
                    start=(k == 0), stop=(k == 1),
                )
            if t % 7 == 0:
                stage[0] = tpool.tile([P, 7, CAT2], bf16, tag="stage", name="stage")
            nc.scalar.activation(out=stage[0][:, t % 7, :], in_=p2[:, 0:CAT2], func=AF.Copy)
            if t % 7 == 6:
                g7 = t // 7
                nc.sync.dma_start(
                    out=OUT[:, g7 * 7 : (g7 + 1) * 7, :], in_=stage[0][:]
                )
    nc.compile()
    return nc


def _build_k3(cpg):
    TPC = RPT * cpg
    NCH = GPC * cpg
    nc = bacc.Bacc("TRN2", target_bir_lowering=False)
    HS = nc.dram_tensor("hs2", [P, NCH, HS2], bf16, kind="ExternalInput")
    ES = nc.dram_tensor("es2", [P, NCH, ES2], bf16, kind="ExternalInput")
    B2 = nc.dram_tensor("bias2", [P, OUT_C], f32, kind="ExternalInput")
    OUT = nc.dram_tensor("out", [P, NT, OUT_C], f32, kind="ExternalOutput")
    with TileContext(nc) as tc, ExitStack() as ctx:
        con = ctx.enter_context(tc.tile_pool(name="con", bufs=1))
        hpool = ctx.enter_context(tc.tile_pool(name="h", bufs=3))
        epool = ctx.enter_context(tc.tile_pool(name="e", bufs=3))
        spool = ctx.enter_context(tc.tile_pool(name="s", bufs=3))
        tpool = ctx.enter_context(tc.tile_pool(name="t", bufs=2))
        psagg = ctx.enter_context(tc.tile_pool(name="pa", bufs=4, space="PSUM"))

        b2t = con.tile([P, OUT_C], f32)
        nc.sync.dma_start(out=b2t[:], in_=B2[:])
        iota_i = con.tile([P, GS], i32)
        nc.gpsimd.iota(iota_i[:], pattern=[[1, GS]], base=0, channel_multiplier=0)
        iota_b = con.tile([P, GS], bf16)
        nc.vector.tensor_copy(out=iota_b[:], in_=iota_i[:])
        iota_e = con.tile([P, GS, TPC], bf16)
        nc.vector.tensor_copy(
            out=iota_e[:], in_=iota_b[:][:, :, None].to_broadcast([P, GS, TPC])
        )

        stage = [None]
        for t in range(NT):
            c0 = t * TPC
            g3 = hpool.tile([P, TPC, HS2], bf16, tag="g3")
            nc.sync.dma_start(out=g3[:], in_=HS[:, c0 : c0 + TPC, :])
            e3 = epool.tile([P, TPC, ES2], bf16, tag="e3")
            nc.sync.dma_start(out=e3[:], in_=ES[:, c0 : c0 + TPC, :])
            dlt = epool.tile([P, TPC], bf16, tag="dl")
            nc.vector.tensor_copy(out=dlt[:], in_=e3[:, :, 2])
            ea = epool.tile([P, TPC], bf16, tag="ea")
            nc.vector.tensor_tensor(
                out=ea[:], in0=e3[:, :, 0], in1=e3[:, :, 1], op=OP.add
            )
            eb = epool.tile([P, TPC], bf16, tag="eb")
            nc.vector.tensor_scalar(
                out=eb[:], in0=ea[:], scalar1=NEG_SLOPE, scalar2=None, op0=OP.mult
            )
            nc.vector.tensor_tensor(out=eb[:], in0=ea[:], in1=eb[:], op=OP.max)
            ee = epool.tile([P, TPC], bf16, tag="ee")
            nc.scalar.activation(out=ee[:], in_=eb[:], func=AF.Exp)
            st2 = spool.tile([P, GS, TPC], bf16, tag="st2")
            nc.vector.tensor_tensor(
                out=st2[:],
                in0=dlt[:][:, None, :].to_broadcast([P, GS, TPC]),
                in1=iota_e[:],
                op=OP.is_equal,
            )
            nc.vector.tensor_tensor(
                out=st2[:],
                in0=st2[:],
                in1=ee[:][:, None, :].to_broadcast([P, GS, TPC]),
                op=OP.mult,
            )
            pagg = psagg.tile([P, 512], f32)
            for r in range(RPT):
                for j in range(cpg):
                    ch = r * cpg + j
                    nc.tensor.matmul(
                        pagg[r * GS : (r + 1) * GS, 0 : OUT_C + 1],
                        lhsT=st2[:, :, ch],
                        rhs=g3[:, ch, 0 : OUT_C + 1],
                        start=(j == 0),
                        stop=(j == cpg - 1),
                        tile_position=(0, r * GS),
                    )
            dent = tpool.tile([P, 1], f32, tag="dent")
            nc.vector.tensor_scalar(
                out=dent[:], in0=pagg[:, OUT_C : OUT_C + 1], scalar1=EPS,
                scalar2=None, op0=OP.add,
            )
            rec = tpool.tile([P, 1], f32, tag="rec")
            nc.vector.reciprocal(out=rec[:], in_=dent[:])
            o = tpool.tile([P, OUT_C], f32, tag="o")
            nc.vector.tensor_tensor(
                out=o[:], in0=pagg[:, 0:OUT_C],
                in1=rec[:].to_broadcast([P, OUT_C]), op=OP.mult,
            )
            nc.vector.tensor_tensor(out=o[:], in0=o[:], in1=b2t[:], op=OP.add)
            if t % 7 == 0:
                stage[0] = tpool.tile([P, 7, OUT_C], f32, tag="stage", name="stage")
            nc.vector.tensor_copy(out=stage[0][:, t % 7, :], in_=o[:])
            if t % 7 == 6:
                g7 = t // 7
                nc.sync.dma_start(
                    out=OUT[:, g7 * 7 : (g7 + 1) * 7, :], in_=stage[0][:]
                )
    nc.compile()
    return nc


_CACHE = {}
TRACE = False
LAST_EXEC_NS = None
LAST_PROFILE = []
RECORD = None  # test-only hook: list to append (tag, nc, core0_map) per launch


def _run(nc, maps, tag):
    global LAST_EXEC_NS
    if RECORD is not None:
        RECORD.append((tag, nc, maps[0]))
    r = run_bass_kernel_spmd(nc, maps, CORE_IDS, trace=TRACE)
    if TRACE:
        ns = r.exec_time_ns
        LAST_PROFILE.append((tag, ns))
        if ns is not None:
            LAST_EXEC_NS = (LAST_EXEC_NS or 0) + ns
    return r.results


def _programs(cpg):
    if cpg not in _CACHE:
        _CACHE[cpg] = (_build_k1(), _build_k2(cpg), _build_k3(cpg))
    return _CACHE[cpg]


def kernel(
    x, edge_index, W1, att_src1, att_dst1, bias1, W2, att_src2, att_dst2, bias2
):
    x = np.asarray(x, np.float32)
    W1 = np.asarray(W1, np.float32)
    W2 = np.asarray(W2, np.float32)
    att_src1 = np.asarray(att_src1, np.float32)
    att_dst1 = np.asarray(att_dst1, np.float32)
    att_src2 = np.asarray(att_src2, np.float32)
    att_dst2 = np.asarray(att_dst2, np.float32)
    bias1 = np.asarray(bias1, np.float32)
    bias2 = np.asarray(bias2, np.float32)

    pre = _prep_graph(np.asarray(edge_index))
    cpg = pre["cpg"]
    nc1, nc2, nc3 = _programs(cpg)
    slot_node = pre["slot_node"]
    node_flat = pre["node_flat"]
    esrc, ednode, edl = pre["esrc"], pre["ednode"], pre["edl"]
    NCH = GPC * cpg

    # ---- K1: h1cat = x @ [W1 c-major | W1@As | W1@Ad] ----
    A_s = np.zeros((C1, HEADS), np.float32)
    A_d = np.zeros((C1, HEADS), np.float32)
    for h in range(HEADS):
        A_s[h * HID : (h + 1) * HID, h] = att_src1[h]
        A_d[h * HID : (h + 1) * HID, h] = att_dst1[h]
    W1cat = np.concatenate([W1[:, PERM_CM], W1 @ A_s, W1 @ A_d], axis=1).astype(BF)
    x_ext = np.concatenate([x, np.zeros((1, IN_C), np.float32)])
    sn = np.where(slot_node < 0, N, slot_node)
    xT = np.ascontiguousarray(x_ext[sn].transpose(0, 2, 1)).astype(BF)  # [8,128,SLOTS]
    r1 = _run(nc1, [{"xT": xT[c], "w1cat": W1cat} for c in range(NCORE)], "k1")
    h1 = np.stack([r1[c]["h1cat"] for c in range(NCORE)])  # [8,P,NT,CAT1]
    h1 = h1.transpose(0, 2, 1, 3).reshape(NCORE * SLOTS, CAT1)  # row c*SLOTS+slot
    h1e = np.concatenate([h1, np.zeros((1, CAT1), h1.dtype)])
    zrow = NCORE * SLOTS

    si = np.where(esrc < 0, zrow, node_flat[np.maximum(esrc, 0)])
    di = np.where(ednode < 0, zrow, node_flat[np.maximum(ednode, 0)])

    # ---- K2 ----
    W2cat = np.concatenate([W2, W2 @ att_src2.T, W2 @ att_dst2.T], axis=1)[PERM_CM]
    w2c = np.concatenate([W2cat[:P], W2cat[P:]], axis=1).astype(BF)  # [128,132]
    b1c = np.tile(bias1[PERM_CM][None], (P, 1)).astype(BF)
    maps2 = []
    for c in range(NCORE):
        hrows = h1e[si[c]]  # [ESLOT, CAT1] bf16
        hs = np.ascontiguousarray(
            hrows[:, 0:C1].reshape(NCH, P, C1).transpose(1, 0, 2)
        )
        es = np.zeros((si.shape[1], ES1), BF)
        es[:, 0:8] = hrows[:, C1 : C1 + 8]
        es[:, 8:16] = h1e[di[c]][:, C1 + 8 : C1 + 16]
        es[:, 16] = edl[c].astype(np.float32)
        es = np.ascontiguousarray(es.reshape(NCH, P, ES1).transpose(1, 0, 2))
        maps2.append({"hs": hs, "es": es, "w2cat": w2c, "bias1": b1c})
    r2 = _run(nc2, maps2, "k2")
    h2 = np.stack([r2[c]["h2cat"] for c in range(NCORE)])
    h2 = h2.transpose(0, 2, 1, 3).reshape(NCORE * SLOTS, CAT2)
    h2e = np.concatenate([h2, np.zeros((1, CAT2), h2.dtype)])

    # ---- K3 ----
    b2c = np.tile(bias2[None], (P, 1)).astype(np.float32)
    maps3 = []
    for c in range(NCORE):
        h2r = h2e[si[c]]
        hs2 = np.zeros((si.shape[1], HS2), BF)
        hs2[:, 0:OUT_C] = h2r[:, 0:OUT_C]
        hs2[:, OUT_C] = 1.0
        es2 = np.zeros((si.shape[1], ES2), BF)
        es2[:, 0] = h2r[:, OUT_C]
        es2[:, 1] = h2e[di[c]][:, OUT_C + 1]
        es2[:, 2] = edl[c].astype(np.float32)
        hs2 = np.ascontiguousarray(hs2.reshape(NCH, P, HS2).transpose(1, 0, 2))
        es2 = np.ascontiguousarray(es2.reshape(NCH, P, ES2).transpose(1, 0, 2))
        maps3.append({"hs2": hs2, "es2": es2, "bias2": b2c})
    r3 = _run(nc3, maps3, "k3")
    out = np.zeros((N, OUT_C), np.float32)
    for c in range(NCORE):
        rows = r3[c]["out"].transpose(1, 0, 2).reshape(SLOTS, OUT_C)
        m = slot_node[c] >= 0
        out[slot_node[c][m]] = rows[m]
    return out
